# revision 11
# baseline (speedup 1.0000x reference)
"""Trainium2 kernel for nn_Dense_RBS_density: rho <- U rho U^T over a batch
of 8 density matrices in the Hamming-weight-2 basis of 32 qubits (dim=496).

The 15 RBS gates act on disjoint qubit pairs, so they commute and fold into a
single orthogonal matrix U (built on host from the 15 angles — negligible
work). In a permuted basis U is block-diagonal with four 124x124 blocks
(each itself made of <=4x4 rotations), so per core (one batch element):
    mm1 bank kt: A^T[mt,kt] = rho'[kt,mt]^T @ B^T[kt,kt]   (16 matmuls)
    mm2 bank mt: out'[mt,kt] = A[mt,kt] @ B^T[kt,kt]       (16 matmuls)

Everything is bf16 (inputs, matmuls, PSUM results, stores): the harness
tolerance (2e-2) dwarfs bf16 rounding (~5e-3), bf16 matmuls stream 1 row per
PE cycle at any p-state (f32r pays 2-4x for 124-wide outputs), and bf16
halves DMA bytes.

Schedule: all four mm1 k-sweeps run back-to-back on the PE while DVE chases
them with PSUM->SBUF at-copies; the four mm2 groups follow; each finished
ps2 bank is DMA'd straight from PSUM to HBM (no SBUF staging, no Activation
copies -> no one-time 1.3us ACT table load). Input is loaded as 4 chunks on
4 different engine DMA queues so the whole load fits in one ~500ns slot.
"""

import itertools
import math

import numpy as np

N_QUBITS = 32
LIST_GATES = [(2 * i, 2 * i + 1) for i in range(15)]
DIM = 496  # C(32, 2)
PT = 124  # partition tile size; 4 * 124 = 496
NT = 4  # number of tiles along each axis
N_CORES = 8
N_WARMUP_MM = 2  # dummy matmuls that ramp the PE clock during the first load
ROW = DIM + PT  # packed input row: 496 rho' columns + 124 block columns


def _gate_pairs():
    """For each gate (a,b), the list of (k, kp) basis-index pairs rotated by
    the gate: k contains a, kp = same state with a replaced by b."""
    pairs = list(itertools.combinations(range(N_QUBITS), 2))
    idx = {p: k for k, p in enumerate(pairs)}
    out = []
    for a, b in LIST_GATES:
        rot = []
        for p, k in idx.items():
            if (a in p) and (b not in p):
                other = p[0] if p[1] == a else p[1]
                kp = idx[tuple(sorted((other, b)))]
                rot.append((k, kp))
        out.append(rot)
    return out


_GATE_PAIRS = _gate_pairs()


def _build_perm():
    """Basis order that block-diagonalizes the folded U: 105 4-blocks (gate
    pair x gate pair), 30 2-blocks (qubit 30/31 partners), 16 fixed states.
    All blocks land inside aligned 124-wide tiles."""
    pairs = list(itertools.combinations(range(N_QUBITS), 2))
    idx = {p: k for k, p in enumerate(pairs)}
    perm = []
    for a in range(15):
        for b in range(a + 1, 15):
            for x in range(2):
                for y in range(2):
                    perm.append(idx[(2 * a + x, 2 * b + y)])
    for q in (30, 31):
        for a in range(15):
            perm.append(idx[tuple(sorted((2 * a, q)))])
            perm.append(idx[tuple(sorted((2 * a + 1, q)))])
    for a in range(15):
        perm.append(idx[(2 * a, 2 * a + 1)])
    perm.append(idx[(30, 31)])
    return np.array(perm)


_PERM = _build_perm()
_INV_PERM = np.argsort(_PERM)


def _build_u(angles: np.ndarray) -> np.ndarray:
    """Fold the 15 commuting RBS gates into one orthogonal DIMxDIM matrix."""
    u = np.eye(DIM, dtype=np.float64)
    for g, rot in enumerate(_GATE_PAIRS):
        c = math.cos(float(angles[g]))
        s = math.sin(float(angles[g]))
        k = np.array([r[0] for r in rot])
        kp = np.array([r[1] for r in rot])
        rk, rkp = u[k].copy(), u[kp].copy()
        u[k] = c * rk + s * rkp
        u[kp] = -s * rk + c * rkp
    return u


_NC_CACHE = {}


def _build_bass():
    import concourse.bass as bass
    import concourse.mybir as mybir
    import concourse.tile as tile
    from concourse.bass import MemorySpace

    mm_dt = mybir.dt.bfloat16

    nc = bass.Bass("TRN2", target_bir_lowering=False, debug=False)
    # 4 chunks of [rho k-tile; U^T k-tile], 124 rows each, bf16.
    inp_d = nc.dram_tensor("inp", [DIM, ROW], mm_dt,
                           kind="ExternalInput").ap()
    out_d = nc.dram_tensor("out", [DIM, DIM], mm_dt,
                           kind="ExternalOutput").ap()

    with tile.TileContext(nc) as tc:
        with (
            tc.tile_pool(name="consts", bufs=1) as consts,
            tc.tile_pool(name="psum", bufs=1, space=MemorySpace.PSUM) as psum,
        ):
            # [124, 4, 620]: per k-tile, 496 rho' columns + this tile's
            # 124x124 diagonal block of B^T.
            inp_sb = consts.tile([PT, NT, ROW], mm_dt, tag="inp")
            at_sb = consts.tile([PT, NT, DIM], mm_dt, tag="at")
            out_sb = consts.tile([PT, NT, DIM], mm_dt, tag="outs")

            dma_is, mm_is, cp_is = [], [], []
            # input chunks on the 3 DMA-capable queues (SP, Act, Pool);
            # chunk 0 (needed first) on SP, chunk 3 (needed last) on Pool
            engs = [nc.sync, nc.scalar, nc.sync, nc.gpsimd]
            for kt in range(NT):
                dma_is.append(engs[kt].dma_start(
                    inp_sb[:, kt, :],
                    inp_d[PT * kt:PT * (kt + 1), :]))

            ps1 = [psum.tile([PT, DIM], mybir.dt.float32, tag=f"ps1_{mt}",
                             name=f"ps1_{mt}") for mt in range(NT)]
            ps2 = [psum.tile([PT, DIM], mybir.dt.float32, tag=f"ps2_{mt}",
                             name=f"ps2_{mt}") for mt in range(NT)]

            # Every matmul is its own PSUM group (the 124-wide output regions
            # are disjoint - no accumulation), so each slice is copyable the
            # moment its single matmul retires: copies chase the PE at 124-col
            # granularity instead of whole banks.
            def mm1_emit(kt, mt):
                mm_is.append(nc.tensor.matmul(
                    ps1[kt][:, mt * PT:(mt + 1) * PT],
                    inp_sb[:, kt, mt * PT:(mt + 1) * PT],
                    inp_sb[:, kt, DIM:ROW],
                    start=True, stop=True,
                ))

            def at_copy(kt, mt, eng):
                sl = slice(mt * PT, (mt + 1) * PT)
                cp_is.append(eng.tensor_copy(at_sb[:, kt, sl], ps1[kt][:, sl]))

            def mm2_emit(mt, kt):
                # needs region (m-band kt, n-band mt) = at tile mt, slice kt
                mm_is.append(nc.tensor.matmul(
                    ps2[mt][:, kt * PT:(kt + 1) * PT],
                    at_sb[:, mt, kt * PT:(kt + 1) * PT],
                    inp_sb[:, kt, DIM:ROW],
                    start=True, stop=True,
                ))

            def out_copy(mt, lo, hi, eng):
                cp_is.append(eng.tensor_copy(out_sb[:, mt, lo * PT:hi * PT],
                                             ps2[mt][:, lo * PT:hi * PT]))

            def out_store(mt, eng):
                dma_is.append(eng.dma_start(
                    out_d[mt * PT:(mt + 1) * PT, :], out_sb[:, mt, :]))

            # mm1 sweeps with Pool chasing every slice (103ns per 124-col
            # slice on Pool, no per-instruction bubble in this model)
            for kt in range(NT):
                for mt in range(NT):
                    mm1_emit(kt, mt)
                    at_copy(kt, mt, nc.gpsimd)

            # mm2 groups; copies per group: Pool takes the early slices
            # fine-grained, DVE takes a trailing half; for the last group the
            # DVE half leads and Pool sweeps the last two slices so the final
            # copy is a 103ns Pool slice. Stores: last one on SP.
            pool, dve = nc.gpsimd, nc.vector
            for g in range(NT):
                for k2 in range(NT):
                    mm2_emit(g, k2)
                    if g < NT - 1:
                        if k2 < 2:
                            out_copy(g, k2, k2 + 1, pool)
                        elif k2 == NT - 1:
                            out_copy(g, 2, 4, dve)
                    else:
                        if k2 == 1:
                            out_copy(g, 0, 2, dve)
                        elif k2 >= 2:
                            out_copy(g, k2, k2 + 1, pool)
                out_store(g, [nc.sync, nc.scalar, nc.scalar, nc.sync][g])

            # Pre-observe every semaphore on SP with single-wait NOPs so the
            # auto-generated kernel-tail Drain needs none of its own.
            for d in dma_is:
                n = nc.sync.nop(nofuse=True)
                tile.add_dep_helper(n.ins, d.ins, True, "pre-drain observe")
            for group in (mm_is, cp_is):
                n = nc.sync.nop(nofuse=True)
                for d in group:
                    tile.add_dep_helper(n.ins, d.ins, True, "pre-drain observe")

    return nc


def _in_maps(input_state: np.ndarray, angles: np.ndarray) -> list[dict]:
    import ml_dtypes

    u = _build_u(np.asarray(angles, np.float64))
    bt = u[_PERM][:, _PERM].T.astype(np.float32)  # B^T, block-diagonal
    rho = np.asarray(input_state, np.float32)[:, _PERM][:, :, _PERM]
    out = []
    for b in range(N_CORES):
        inp = np.empty((DIM, ROW), ml_dtypes.bfloat16)
        inp[:, :DIM] = rho[b]
        for kt in range(NT):
            band = slice(kt * PT, (kt + 1) * PT)
            inp[band, DIM:] = bt[band, band]
        out.append({"inp": inp})
    return out


def kernel(input_state: np.ndarray, angles: np.ndarray) -> np.ndarray:
    from concourse.bass_utils import run_bass_kernel_spmd

    if "nc" not in _NC_CACHE:
        _NC_CACHE["nc"] = _build_bass()
    nc = _NC_CACHE["nc"]

    in_maps = _in_maps(input_state, angles)
    res = run_bass_kernel_spmd(nc, in_maps, core_ids=list(range(N_CORES)))
    out = np.stack([np.asarray(res.results[b]["out"], np.float32)
                    for b in range(N_CORES)], axis=0)
    out = np.ascontiguousarray(out[:, _INV_PERM][:, :, _INV_PERM])
    return out.astype(np.float32)


# revision 15
# speedup vs baseline: 1.0871x; 1.0871x over previous
"""Trainium2 kernel for nn_Dense_RBS_density: rho <- U rho U^T over a batch
of 8 density matrices in the Hamming-weight-2 basis of 32 qubits (dim=496).

The 15 RBS gates act on disjoint qubit pairs, so they commute and fold into a
single orthogonal matrix U (built on host from the 15 angles — negligible
work). In a permuted basis U is block-diagonal with four 124x124 blocks
(each itself made of <=4x4 rotations), so per core (one batch element):
    mm1 bank kt: A^T[mt,kt] = rho'[kt,mt]^T @ B^T[kt,kt]   (16 matmuls)
    mm2 bank mt: out'[mt,kt] = A[mt,kt] @ B^T[kt,kt]       (16 matmuls)

Everything is bf16 (inputs, matmuls, PSUM results, stores): the harness
tolerance (2e-2) dwarfs bf16 rounding (~5e-3), bf16 matmuls stream 1 row per
PE cycle at any p-state (f32r pays 2-4x for 124-wide outputs), and bf16
halves DMA bytes.

Schedule: all four mm1 k-sweeps run back-to-back on the PE while DVE chases
them with PSUM->SBUF at-copies; the four mm2 groups follow; each finished
ps2 bank is DMA'd straight from PSUM to HBM (no SBUF staging, no Activation
copies -> no one-time 1.3us ACT table load). Input is loaded as 4 chunks on
4 different engine DMA queues so the whole load fits in one ~500ns slot.
"""

import itertools
import math

import numpy as np

N_QUBITS = 32
LIST_GATES = [(2 * i, 2 * i + 1) for i in range(15)]
DIM = 496  # C(32, 2)
PT = 124  # partition tile size; 4 * 124 = 496
NT = 4  # number of tiles along each axis
N_CORES = 8
N_WARMUP_MM = 1  # dummy matmul that delays the PE past the chunk-0 release
ROW = DIM + PT  # packed input row: 496 rho' columns + 124 block columns


def _gate_pairs():
    """For each gate (a,b), the list of (k, kp) basis-index pairs rotated by
    the gate: k contains a, kp = same state with a replaced by b."""
    pairs = list(itertools.combinations(range(N_QUBITS), 2))
    idx = {p: k for k, p in enumerate(pairs)}
    out = []
    for a, b in LIST_GATES:
        rot = []
        for p, k in idx.items():
            if (a in p) and (b not in p):
                other = p[0] if p[1] == a else p[1]
                kp = idx[tuple(sorted((other, b)))]
                rot.append((k, kp))
        out.append(rot)
    return out


_GATE_PAIRS = _gate_pairs()


def _build_perm():
    """Basis order that block-diagonalizes the folded U: 105 4-blocks (gate
    pair x gate pair), 30 2-blocks (qubit 30/31 partners), 16 fixed states.
    All blocks land inside aligned 124-wide tiles."""
    pairs = list(itertools.combinations(range(N_QUBITS), 2))
    idx = {p: k for k, p in enumerate(pairs)}
    perm = []
    for a in range(15):
        for b in range(a + 1, 15):
            for x in range(2):
                for y in range(2):
                    perm.append(idx[(2 * a + x, 2 * b + y)])
    for q in (30, 31):
        for a in range(15):
            perm.append(idx[tuple(sorted((2 * a, q)))])
            perm.append(idx[tuple(sorted((2 * a + 1, q)))])
    for a in range(15):
        perm.append(idx[(2 * a, 2 * a + 1)])
    perm.append(idx[(30, 31)])
    return np.array(perm)


_PERM = _build_perm()
_INV_PERM = np.argsort(_PERM)


def _build_u(angles: np.ndarray) -> np.ndarray:
    """Fold the 15 commuting RBS gates into one orthogonal DIMxDIM matrix."""
    u = np.eye(DIM, dtype=np.float64)
    for g, rot in enumerate(_GATE_PAIRS):
        c = math.cos(float(angles[g]))
        s = math.sin(float(angles[g]))
        k = np.array([r[0] for r in rot])
        kp = np.array([r[1] for r in rot])
        rk, rkp = u[k].copy(), u[kp].copy()
        u[k] = c * rk + s * rkp
        u[kp] = -s * rk + c * rkp
    return u


_NC_CACHE = {}


def _build_bass():
    import concourse.bass as bass
    import concourse.mybir as mybir
    import concourse.tile as tile
    from concourse.bass import MemorySpace

    mm_dt = mybir.dt.bfloat16

    nc = bass.Bass("TRN2", target_bir_lowering=False, debug=False)
    # 4 chunks of [rho k-tile; U^T k-tile], 124 rows each, bf16.
    inp_d = nc.dram_tensor("inp", [DIM, ROW], mm_dt,
                           kind="ExternalInput").ap()
    out_d = nc.dram_tensor("out", [DIM, DIM], mm_dt,
                           kind="ExternalOutput").ap()

    with tile.TileContext(nc) as tc:
        with (
            tc.tile_pool(name="consts", bufs=1) as consts,
            tc.tile_pool(name="psum", bufs=1, space=MemorySpace.PSUM) as psum,
        ):
            # [124, 4, 620]: per k-tile, 496 rho' columns + this tile's
            # 124x124 diagonal block of B^T.
            inp_sb = consts.tile([PT, NT, ROW], mm_dt, tag="inp")
            at_sb = consts.tile([PT, NT, DIM], mm_dt, tag="at")
            out_sb = consts.tile([PT, NT, DIM], mm_dt, tag="outs")
            warm_sb = consts.tile([PT, DIM], mm_dt, tag="warm")

            dma_is, mm_is, cp_is = [], [], []
            # In this simulator a consumer that parks on a DMA semaphore
            # before the DMA's engine-release pays the full ~1.7us DMA init
            # latency; one that checks after release proceeds immediately.
            # So every DMA-sem consumer must ARRIVE LATE: the PE runs one
            # small warmup matmul (gated on a DVE memset) so it reaches the
            # chunk-0 wait after the load's 500ns engine slot has drained.
            warm_i = nc.vector.memset(warm_sb, 0.0)
            # input chunks: chunk 0 on Pool (SWDGE dispatches at t=100,
            # before the start barrier), the rest on SP/Act HWDGE queues
            engs = [nc.gpsimd, nc.sync, nc.scalar, nc.sync]
            for kt in range(NT):
                dma_is.append(engs[kt].dma_start(
                    inp_sb[:, kt, :],
                    inp_d[PT * kt:PT * (kt + 1), :]))

            ps1 = [psum.tile([PT, DIM], mybir.dt.float32, tag=f"ps1_{mt}",
                             name=f"ps1_{mt}") for mt in range(NT)]
            ps2 = [psum.tile([PT, DIM], mybir.dt.float32, tag=f"ps2_{mt}",
                             name=f"ps2_{mt}") for mt in range(NT)]
            for _ in range(N_WARMUP_MM):
                mm_is.append(nc.tensor.matmul(
                    ps2[0][:, :PT], warm_sb[:, :PT], warm_sb[:, :PT],
                    start=True, stop=True))

            # Every matmul is its own PSUM group (the 124-wide output regions
            # are disjoint - no accumulation), so each slice is copyable the
            # moment its single matmul retires: copies chase the PE at 124-col
            # granularity instead of whole banks.
            def mm1_emit(kt, mt):
                mm_is.append(nc.tensor.matmul(
                    ps1[kt][:, mt * PT:(mt + 1) * PT],
                    inp_sb[:, kt, mt * PT:(mt + 1) * PT],
                    inp_sb[:, kt, DIM:ROW],
                    start=True, stop=True,
                ))

            def at_copy(kt, mt, eng):
                sl = slice(mt * PT, (mt + 1) * PT)
                cp_is.append(eng.tensor_copy(at_sb[:, kt, sl], ps1[kt][:, sl]))

            def mm2_emit(mt, kt):
                # needs region (m-band kt, n-band mt) = at tile mt, slice kt
                mm_is.append(nc.tensor.matmul(
                    ps2[mt][:, kt * PT:(kt + 1) * PT],
                    at_sb[:, mt, kt * PT:(kt + 1) * PT],
                    inp_sb[:, kt, DIM:ROW],
                    start=True, stop=True,
                ))

            def out_copy(mt, lo, hi, eng):
                cp_is.append(eng.tensor_copy(out_sb[:, mt, lo * PT:hi * PT],
                                             ps2[mt][:, lo * PT:hi * PT]))

            def out_store(mt, eng):
                dma_is.append(eng.dma_start(
                    out_d[mt * PT:(mt + 1) * PT, :], out_sb[:, mt, :]))

            # mm1 sweeps with Pool chasing every slice (103ns per 124-col
            # slice on Pool, no per-instruction bubble in this model)
            for kt in range(NT):
                for mt in range(NT):
                    mm1_emit(kt, mt)
                    at_copy(kt, mt, nc.gpsimd)

            # mm2 groups; copies per group: Pool takes the early slices
            # fine-grained, DVE takes trailing halves; for the last group the
            # DVE half leads and Pool sweeps the last two slices so the final
            # copy is a 103ns Pool slice. Stores: last one on SP.
            pool, dve = nc.gpsimd, nc.vector
            for g in range(NT):
                for k2 in range(NT):
                    mm2_emit(g, k2)
                    if g < NT - 1:
                        if k2 < 2:
                            out_copy(g, k2, k2 + 1, pool)
                        elif k2 == NT - 1:
                            out_copy(g, 2, 4, dve)
                    else:
                        if k2 == 1:
                            out_copy(g, 0, 2, dve)
                        elif k2 >= 2:
                            out_copy(g, k2, k2 + 1, pool)
                out_store(g, [nc.sync, nc.scalar, nc.scalar, nc.sync][g])

            # Pre-observe every DMA on its OWN engine (queue order makes the
            # wait instant - a cross-engine observer would park on the DMA
            # sem and eat the ~1.7us DMA-completion latency), and each engine
            # group on SP, so the auto-generated kernel-tail Drain and final
            # barrier carry no waits of their own.
            eng_of = {mybir.EngineType.SP: nc.sync,
                      mybir.EngineType.Activation: nc.scalar,
                      mybir.EngineType.Pool: nc.gpsimd,
                      mybir.EngineType.DVE: nc.vector,
                      mybir.EngineType.PE: nc.tensor}
            for d in dma_is:
                n = eng_of[d.ins.engine].nop(nofuse=True)
                tile.add_dep_helper(n.ins, d.ins, True, "pre-drain observe")
            for group in (mm_is, cp_is, [warm_i]):
                n = nc.sync.nop(nofuse=True)
                for d in group:
                    tile.add_dep_helper(n.ins, d.ins, True, "pre-drain observe")

    return nc


def _in_maps(input_state: np.ndarray, angles: np.ndarray) -> list[dict]:
    import ml_dtypes

    u = _build_u(np.asarray(angles, np.float64))
    bt = u[_PERM][:, _PERM].T.astype(np.float32)  # B^T, block-diagonal
    rho = np.asarray(input_state, np.float32)[:, _PERM][:, :, _PERM]
    out = []
    for b in range(N_CORES):
        inp = np.empty((DIM, ROW), ml_dtypes.bfloat16)
        inp[:, :DIM] = rho[b]
        for kt in range(NT):
            band = slice(kt * PT, (kt + 1) * PT)
            inp[band, DIM:] = bt[band, band]
        out.append({"inp": inp})
    return out


def kernel(input_state: np.ndarray, angles: np.ndarray) -> np.ndarray:
    from concourse.bass_utils import run_bass_kernel_spmd

    if "nc" not in _NC_CACHE:
        _NC_CACHE["nc"] = _build_bass()
    nc = _NC_CACHE["nc"]

    in_maps = _in_maps(input_state, angles)
    res = run_bass_kernel_spmd(nc, in_maps, core_ids=list(range(N_CORES)))
    out = np.stack([np.asarray(res.results[b]["out"], np.float32)
                    for b in range(N_CORES)], axis=0)
    out = np.ascontiguousarray(out[:, _INV_PERM][:, :, _INV_PERM])
    return out.astype(np.float32)
